# revision 1
# baseline (speedup 1.0000x reference)
"""Llama GQA attention (B=1, S=2048, H=4096, 32 heads / 8 KV heads, RoPE, causal)
as a tensor-parallel Bass/Tile kernel on 8 Trainium2 NeuronCores.

Sharding: core c computes Q heads [4c, 4c+4) and KV head c (GQA groups align),
full causal attention for those heads, then AllGathers the (transposed)
attention outputs and computes output features [512c, 512c+512) of o_proj.
The AllGather is chunked per 512-seq block and overlapped with attention of
later chunks; o_proj chunk j consumes only AllGather chunk j.

All activations live transposed ([feat, seq]) so every matmul contracts on the
partition axis:
  qT = (Wq/sqrt(hd)) @ xT          [128, 4, 2048]   (RoPE applied from PSUM)
  kT = Wk @ xT                     [128, 2048]      (RoPE applied from PSUM)
  vT = Wv @ xT -> PE-transpose ->  v [2048, 128]    (natural, for PV lhsT)
  sT_blk = K_blk @ qT_chunk        [128 k, 512 q]   (PSUM)
  pT_blk = exp(sT_blk)             bf16, causal-masked via affine_select
  oT    += V_blk.T @ pT_blk        [128 hd, 512 q]  (PSUM accum over k blocks)
  denom  = ones.T @ sum_blk(pT)    [1, 512]
  attT   = oT * (1/denom)          -> AllGather_j -> [4096, 512] per chunk
  outT_j = Wo_slice @ attT_full_j  [512, 512] fp32

build_nc(collective=False, reps=N) builds a single-core benchmarking variant:
the AllGather becomes local DMA copies and the whole pipeline repeats N times
under a Tile For_i loop, so device time can be measured as a wall-clock slope.
"""
import numpy as np
import ml_dtypes
from contextlib import ExitStack

import concourse.bass as bass
import concourse.mybir as mybir
import concourse.tile as tile
from concourse import bacc
from concourse.bass import ts, ds
from concourse.masks import make_identity

N_CORES = 8
S = 2048
HIDDEN = 4096
NUM_HEADS = 32
NUM_KV_HEADS = 8
HEAD_DIM = 128
HEADS_PER_CORE = NUM_HEADS // N_CORES          # 4
QSLICE = HEADS_PER_CORE * HEAD_DIM             # 512
KT = HIDDEN // 128                             # 32 contraction tiles
SC = S // 512                                  # 4 seq chunks of 512
ROPE_THETA = 10000.0

F32 = mybir.dt.float32
BF16 = mybir.dt.bfloat16
I32 = mybir.dt.int32

TWO_PI = float(2 * np.pi)
PI = float(np.pi)
HALF_PI = float(np.pi / 2)

_cache = {}


def _build_trig_chunk(nc, tt_f32, tt_i32, posb, invf_t, s, shift, out_tile):
    """out_tile[:, :512] = sin(pos[sc]*invf + shift), range-reduced to [-pi, pi]."""
    fr = tt_f32.tile([128, 512], F32, tag="ttf", name="fr")
    nc.vector.tensor_scalar_mul(fr[:], posb[:, ts(s, 512)], invf_t[:])
    if shift != 0.0:
        nc.vector.tensor_scalar_add(fr[:], fr[:], shift)
    u = tt_f32.tile([128, 512], F32, tag="ttf", name="u")
    nc.vector.tensor_scalar_mul(u[:], fr[:], float(1.0 / TWO_PI))
    ui = tt_i32.tile([128, 512], I32, tag="tti", name="ui")
    nc.vector.tensor_copy(ui[:], u[:])
    uf = tt_f32.tile([128, 512], F32, tag="ttf", name="uf")
    nc.vector.tensor_copy(uf[:], ui[:])
    nc.vector.tensor_scalar_mul(uf[:], uf[:], -TWO_PI)
    nc.vector.tensor_add(fr[:], fr[:], uf[:])
    # correction pass: fr -= 2pi*(fr > pi); fr += 2pi*(fr < -pi)
    g = tt_f32.tile([128, 512], F32, tag="ttf", name="g")
    nc.vector.tensor_scalar(g[:], fr[:], PI, -TWO_PI,
                            mybir.AluOpType.is_gt, mybir.AluOpType.mult)
    nc.vector.tensor_add(fr[:], fr[:], g[:])
    nc.vector.tensor_scalar(g[:], fr[:], -PI, TWO_PI,
                            mybir.AluOpType.is_lt, mybir.AluOpType.mult)
    nc.vector.tensor_add(fr[:], fr[:], g[:])
    nc.scalar.activation(out_tile, fr[:], mybir.ActivationFunctionType.Sin)
    return out_tile


def build_nc(collective=True, reps=1):
    assert collective is False or reps == 1
    nc = bacc.Bacc("TRN2", target_bir_lowering=False, debug=False,
                   num_devices=N_CORES if collective else 1)
    xT = nc.dram_tensor("xT", [HIDDEN, S], BF16, kind="ExternalInput").ap()
    wqT = nc.dram_tensor("wqT", [HIDDEN, QSLICE], BF16, kind="ExternalInput").ap()
    wkT = nc.dram_tensor("wkT", [HIDDEN, HEAD_DIM], BF16, kind="ExternalInput").ap()
    wvT = nc.dram_tensor("wvT", [HIDDEN, HEAD_DIM], BF16, kind="ExternalInput").ap()
    woT = nc.dram_tensor("woT", [HIDDEN, QSLICE], BF16, kind="ExternalInput").ap()
    pos = nc.dram_tensor("pos", [1, S], I32, kind="ExternalInput").ap()
    invf = nc.dram_tensor("invf", [128, 1], F32, kind="ExternalInput").ap()
    outT = nc.dram_tensor("outT", [QSLICE, S], F32, kind="ExternalOutput").ap()

    xT_r = xT.rearrange("(kt p) s -> p kt s", p=128)
    wqT_r = wqT.rearrange("(kt p) m -> p kt m", p=128)
    wkT_r = wkT.rearrange("(kt p) m -> p kt m", p=128)
    wvT_r = wvT.rearrange("(kt p) m -> p kt m", p=128)
    woT_r = woT.rearrange("(kt p) m -> p kt m", p=128)

    with tile.TileContext(nc) as tc, ExitStack() as ctx:
        const = ctx.enter_context(tc.tile_pool(name="const", bufs=1))
        bigw = ctx.enter_context(tc.tile_pool(name="bigw", bufs=1))
        slab = ctx.enter_context(tc.tile_pool(name="slab", bufs=2))
        trig = ctx.enter_context(tc.tile_pool(name="trig", bufs=2))
        tt_f32 = ctx.enter_context(tc.tile_pool(name="ttf", bufs=3))
        tt_i32 = ctx.enter_context(tc.tile_pool(name="tti", bufs=1))
        ppool = ctx.enter_context(tc.tile_pool(name="ppool", bufs=6))
        f32a = ctx.enter_context(tc.tile_pool(name="f32a", bufs=3))
        small = ctx.enter_context(tc.tile_pool(name="small", bufs=2))
        dram = ctx.enter_context(tc.tile_pool(name="dram", bufs=1, space="DRAM"))
        psum = ctx.enter_context(tc.tile_pool(name="psum", bufs=2, space="PSUM"))
        psum_d = ctx.enter_context(tc.tile_pool(name="psum_d", bufs=2, space="PSUM"))
        psum_t = ctx.enter_context(tc.tile_pool(name="psum_t", bufs=1, space="PSUM"))
        psum_s = ctx.enter_context(tc.tile_pool(name="psum_s", bufs=3, space="PSUM"))

        # ---- persistent constants
        invf_t = const.tile([128, 1], F32)
        nc.sync.dma_start(invf_t[:], invf[:])
        ones_t = const.tile([128, 1], BF16)
        nc.vector.memset(ones_t[:], 1.0)
        ident = const.tile([128, 128], BF16)
        make_identity(nc, ident[:])
        pos_i = const.tile([1, S], I32)
        nc.sync.dma_start(pos_i[:], pos[:])
        pos_f = const.tile([1, S], F32)
        nc.vector.tensor_copy(pos_f[:], pos_i[:])
        posb = const.tile([128, S], F32)
        nc.gpsimd.partition_broadcast(posb[:], pos_f[:])

        # causal mask tiles: mask_d[k, q] = (q - 128d - k >= 0)
        masks_t = const.tile([128, 4, 512], BF16, name="masks_t")
        nc.gpsimd.memset(masks_t[:], 1.0)
        for d in range(4):
            nc.gpsimd.affine_select(
                masks_t[:, d, :], masks_t[:, d, :], pattern=[[1, 512]],
                compare_op=mybir.AluOpType.is_ge, fill=0.0,
                base=-128 * d, channel_multiplier=-1)

        qT_sb = const.tile([128, HEADS_PER_CORE, S], BF16)         # 16KB/part
        kT_sb = const.tile([128, S], BF16)                         # 4KB/part
        v_sb = const.tile([128, S // 128, HEAD_DIM], BF16)         # 4KB/part
        wk_sb = const.tile([128, KT, HEAD_DIM], BF16)
        wv_sb = const.tile([128, KT, HEAD_DIM], BF16)

        ag_ins = [dram.tile([QSLICE, 512], BF16, tag=f"agin{j}",
                            name=f"agin{j}") for j in range(SC)]
        if collective:
            ag_outs = [dram.tile([NUM_HEADS * HEAD_DIM, 512], BF16,
                                 addr_space="Shared", tag=f"agout{j}",
                                 name=f"agout{j}") for j in range(SC)]
        else:
            ag_outs = [dram.tile([NUM_HEADS * HEAD_DIM, 512], BF16,
                                 tag=f"agout{j}", name=f"agout{j}")
                       for j in range(SC)]

        def emit_body(first):
            # per-iteration big weights: wq, then wo reusing the same slot
            wq_sb = bigw.tile([128, KT, QSLICE], BF16, tag="bigw", name="wq_sb")
            x_slab0 = slab.tile([128, KT, 512], BF16, tag="slab", name="x_slab0")
            for q in range(8):  # chunked so ktile 0 lands early
                kts = ds(4 * q, 4)
                if first:
                    nc.sync.dma_start(wv_sb[:, kts, :], wvT_r[:, kts, :])
                    nc.sync.dma_start(wk_sb[:, kts, :], wkT_r[:, kts, :])
                nc.sync.dma_start(x_slab0[:, kts, :], xT_r[:, kts, 0:512])
                nc.sync.dma_start(wq_sb[:, kts, :], wqT_r[:, kts, :])

            # ---- phase 1: projections + RoPE, per 512-seq chunk
            for s in range(SC):
                if s == 0:
                    x_slab = x_slab0
                else:
                    x_slab = slab.tile([128, KT, 512], BF16, tag="slab",
                                       name="x_slab")
                    nc.sync.dma_start(x_slab[:], xT_r[:, :, ts(s, 512)])

                # sin/cos for this chunk (signed sin: top half negated)
                sin_c = trig.tile([128, 512], F32, tag="sin", name="sin_c")
                _build_trig_chunk(nc, tt_f32, tt_i32, posb, invf_t, s, 0.0,
                                  sin_c[:])
                nc.vector.tensor_scalar_mul(sin_c[0:64, :], sin_c[0:64, :], -1.0)
                cos_c = trig.tile([128, 512], F32, tag="cos", name="cos_c")
                _build_trig_chunk(nc, tt_f32, tt_i32, posb, invf_t, s, HALF_PI,
                                  cos_c[:])

                # V first: compute vT [128 hd, 512 seq], PE-transpose to natural
                pvt = psum_s.tile([128, 512], F32, tag="s", name="pvt")
                for kt in range(KT):
                    nc.tensor.matmul(pvt[:], wv_sb[:, kt, :], x_slab[:, kt, :],
                                     start=(kt == 0), stop=(kt == KT - 1))
                vt_c = small.tile([128, 512], BF16, tag="vt", name="vt_c")
                nc.scalar.copy(vt_c[:], pvt[:])
                for t in range(4):
                    ptr = psum_t.tile([128, 128], BF16, tag="t", name="ptr")
                    nc.tensor.transpose(ptr[:], vt_c[:, ts(t, 128)], ident[:])
                    nc.scalar.copy(v_sb[:, 4 * s + t, :], ptr[:])

                # K then Q heads: [128 out, 512 seq] PSUM, fused RoPE -> SBUF
                for t in [HEADS_PER_CORE, 0, 1, 2, 3]:
                    pq = psum_s.tile([128, 512], F32, tag="s", name="pq")
                    for kt in range(KT):
                        lhsT = (wq_sb[:, kt, ts(t, 128)] if t < HEADS_PER_CORE
                                else wk_sb[:, kt, :])
                        nc.tensor.matmul(pq[:], lhsT, x_slab[:, kt, :],
                                         start=(kt == 0), stop=(kt == KT - 1))
                    rot = f32a.tile([128, 512], F32, tag="f32a", name="rot")
                    nc.vector.tensor_tensor(rot[0:64, :], pq[64:128, :],
                                            sin_c[0:64, :], mybir.AluOpType.mult)
                    nc.vector.tensor_tensor(rot[64:128, :], pq[0:64, :],
                                            sin_c[64:128, :], mybir.AluOpType.mult)
                    cq = f32a.tile([128, 512], F32, tag="f32a", name="cq")
                    nc.vector.tensor_tensor(cq[:], pq[:], cos_c[:],
                                            mybir.AluOpType.mult)
                    dest = (qT_sb[:, t, ts(s, 512)] if t < HEADS_PER_CORE
                            else kT_sb[:, ts(s, 512)])
                    nc.vector.tensor_tensor(dest, cq[:], rot[:],
                                            mybir.AluOpType.add)

            # wo shares wq's slot; DMA starts once the last proj matmul retires
            wo_sb = bigw.tile([128, KT, QSLICE], BF16, tag="bigw", name="wo_sb")
            nc.sync.dma_start(wo_sb[:], woT_r[:])

            # ---- phase 2: attention, chunk-major so AllGather_j overlaps j+1
            for j in range(SC):
                nblk = 4 * (j + 1)
                for h in range(HEADS_PER_CORE):
                    po = psum.tile([128, 512], F32, tag="o", name="po")
                    pd = psum_d.tile([1, 512], F32, tag="d", name="pd")

                    def emit_scores(ki, h=h, j=j):
                        ps_ = psum_s.tile([128, 512], F32, tag="s", name="ps_")
                        nc.tensor.matmul(ps_[:], kT_sb[:, ts(ki, 128)],
                                         qT_sb[:, h, ts(j, 512)],
                                         start=True, stop=True)
                        pT = ppool.tile([128, 512], BF16, tag="pT", name="pT")
                        nc.scalar.activation(pT[:], ps_[:],
                                             mybir.ActivationFunctionType.Exp)
                        d = ki - 4 * j
                        if d >= 0:  # diagonal block: causal mask q-k-128d >= 0
                            nc.vector.tensor_tensor(pT[:], pT[:],
                                                    masks_t[:, d, :],
                                                    mybir.AluOpType.mult)
                        return pT

                    # scores emitted 2 blocks ahead of PV (SW pipeline);
                    # denominator accumulates on PE via ones-matmuls into pd
                    DEPTH = 2
                    pts = [emit_scores(kk) for kk in range(min(DEPTH, nblk))]
                    for ki in range(nblk):
                        pT = pts[ki]
                        if ki + DEPTH < nblk:
                            pts.append(emit_scores(ki + DEPTH))
                        nc.tensor.matmul(po[:], v_sb[:, ki, :], pT[:],
                                         start=(ki == 0), stop=(ki == nblk - 1))
                        nc.tensor.matmul(pd[:], ones_t[:], pT[:],
                                         start=(ki == 0), stop=(ki == nblk - 1))
                    recip = small.tile([1, 512], F32, tag="recip", name="recip")
                    nc.vector.reciprocal(recip[:], pd[:])
                    rb = f32a.tile([128, 512], F32, tag="f32a", name="rb")
                    nc.gpsimd.partition_broadcast(rb[:], recip[:])
                    att = small.tile([128, 512], BF16, tag="att", name="att")
                    nc.vector.tensor_tensor(att[:], po[:], rb[:],
                                            mybir.AluOpType.mult)
                    nc.sync.dma_start(ag_ins[j][ts(h, 128), :], att[:])

                if collective:
                    nc.gpsimd.collective_compute(
                        "AllGather", mybir.AluOpType.bypass,
                        replica_groups=[list(range(N_CORES))],
                        ins=[ag_ins[j].opt()], outs=[ag_outs[j].opt()],
                    )
                else:
                    for r in range(N_CORES):
                        nc.sync.dma_start(ag_outs[j][ds(r * QSLICE, QSLICE), :],
                                          ag_ins[j][:])

            # ---- phase 3: o_proj per seq chunk (chunk j needs AllGather j only)
            for s in range(SC):
                ag_r = ag_outs[s].rearrange("(kt p) s -> p kt s", p=128)
                a_slab = slab.tile([128, KT, 512], BF16, tag="slab",
                                   name="a_slab")
                nc.sync.dma_start(a_slab[:], ag_r[:])
                for ft in range(QSLICE // 128):
                    pq = psum_s.tile([128, 512], F32, tag="s", name="pq_o")
                    for kt in range(KT):
                        nc.tensor.matmul(pq[:], wo_sb[:, kt, ts(ft, 128)],
                                         a_slab[:, kt, :],
                                         start=(kt == 0), stop=(kt == KT - 1))
                    ot = f32a.tile([128, 512], F32, tag="f32a", name="ot")
                    nc.scalar.copy(ot[:], pq[:])
                    nc.sync.dma_start(outT[ts(ft, 128), ts(s, 512)], ot[:])

        if reps == 1:
            emit_body(first=True)
        else:
            emit_body(first=True)
            with tc.For_i(0, reps - 1, 1):
                emit_body(first=False)

    nc.finalize()
    return nc


def _prep_inputs(hidden_states, Wq, Wk, Wv, Wo, position_ids):
    """Slice/cast per-core inputs (host-side layout prep only)."""
    bf = ml_dtypes.bfloat16
    x = np.ascontiguousarray(np.asarray(hidden_states, np.float32)[0].T).astype(bf)
    scale = 1.0 / np.sqrt(HEAD_DIM)
    invf_half = (1.0 / (ROPE_THETA ** (np.arange(0, HEAD_DIM, 2, dtype=np.float64)
                                       / HEAD_DIM))).astype(np.float32)
    invf_np = np.concatenate([invf_half, invf_half])[:, None].astype(np.float32)
    pos_np = np.asarray(position_ids, np.int32).reshape(1, S)
    in_maps = []
    for c in range(N_CORES):
        wq_c = (np.asarray(Wq, np.float32)[c * QSLICE:(c + 1) * QSLICE] * scale)
        wk_c = np.asarray(Wk, np.float32)[c * HEAD_DIM:(c + 1) * HEAD_DIM]
        wv_c = np.asarray(Wv, np.float32)[c * HEAD_DIM:(c + 1) * HEAD_DIM]
        wo_c = np.asarray(Wo, np.float32)[c * QSLICE:(c + 1) * QSLICE]
        in_maps.append({
            "xT": x,
            "wqT": np.ascontiguousarray(wq_c.T).astype(bf),
            "wkT": np.ascontiguousarray(wk_c.T).astype(bf),
            "wvT": np.ascontiguousarray(wv_c.T).astype(bf),
            "woT": np.ascontiguousarray(wo_c.T).astype(bf),
            "pos": pos_np,
            "invf": invf_np,
        })
    return in_maps


def kernel(hidden_states, Wq, Wk, Wv, Wo, position_ids):
    from concourse.bass_utils import run_bass_kernel_spmd
    if "nc" not in _cache:
        _cache["nc"] = build_nc()
    nc = _cache["nc"]
    in_maps = _prep_inputs(hidden_states, Wq, Wk, Wv, Wo, position_ids)
    res = run_bass_kernel_spmd(nc, in_maps, core_ids=list(range(N_CORES)))
    out = np.concatenate([res.results[c]["outT"].T for c in range(N_CORES)], axis=1)
    return out[None].astype(np.float32)



# revision 9
# speedup vs baseline: 1.1407x; 1.1407x over previous
"""Llama GQA attention (B=1, S=2048, H=4096, 32 heads / 8 KV heads, RoPE, causal)
as a tensor-parallel Bass/Tile kernel on 8 Trainium2 NeuronCores.

Sharding: core c computes Q heads [4c, 4c+4) and KV head c (GQA groups align),
full causal attention for those heads, then AllGathers the (transposed)
attention outputs and computes output features [512c, 512c+512) of o_proj.

v2 vs baseline:
- cos/sin precomputed host-side (DMA'd in), killing all on-device trig.
- Causal mask folded into the scores PSUM via an identity-matmul of a -1e9
  bias tile (PE, cheap) instead of DVE mask multiplies after exp.
- exp batched 2 k-blocks per ACT instruction ([128,2,512] PSUM pairs).
- Softmax denominator: bf16 DVE accumulation of pT blocks + one ones-matmul
  per (head, chunk); reciprocal via reciprocal_approx_fast on a PE-broadcast
  [128,512] tile (f32r outer product). gpsimd queue carries ONLY AllGather
  triggers so collectives never block normalization work.
- Projections and attention interleaved per 512-seq chunk so AllGather j
  is in flight while proj/attention of later chunks keeps the PE warm
  (HAM clock gate: PE only reaches 2.4 GHz with no idle gaps).
- o_proj emitted at the tail, consuming AllGather chunks that completed long
  before; feature-pairs batched into [128,2,512] PSUM tiles.

All activations live transposed ([feat, seq]); every matmul contracts on the
partition axis.
"""
import numpy as np
import ml_dtypes
from contextlib import ExitStack

import concourse.bass as bass
import concourse.mybir as mybir
import concourse.tile as tile
from concourse import bacc
from concourse.bass import ts, ds
from concourse.masks import make_identity

N_CORES = 8
S = 2048
HIDDEN = 4096
NUM_HEADS = 32
HEAD_DIM = 128
HEADS_PER_CORE = NUM_HEADS // N_CORES          # 4
QSLICE = HEADS_PER_CORE * HEAD_DIM             # 512
KT = HIDDEN // 128                             # 32 contraction tiles
SC = S // 512                                  # 4 seq chunks of 512
ROPE_THETA = 10000.0
NEG = -1.0e9

F32 = mybir.dt.float32
F32R = mybir.dt.float32r
BF16 = mybir.dt.bfloat16

_cache = {}


def build_nc():
    nc = bacc.Bacc("TRN2", target_bir_lowering=False, debug=False,
                   num_devices=N_CORES)
    xT = nc.dram_tensor("xT", [HIDDEN, S], BF16, kind="ExternalInput").ap()
    wqT = nc.dram_tensor("wqT", [HIDDEN, QSLICE], BF16, kind="ExternalInput").ap()
    wkvT = nc.dram_tensor("wkvT", [HIDDEN, 2 * HEAD_DIM], BF16,
                          kind="ExternalInput").ap()
    woT = nc.dram_tensor("woT", [HIDDEN, QSLICE], BF16, kind="ExternalInput").ap()
    cosT = nc.dram_tensor("cosT", [HEAD_DIM, S], F32, kind="ExternalInput").ap()
    sinT = nc.dram_tensor("sinT", [HEAD_DIM, S], F32, kind="ExternalInput").ap()
    outT = nc.dram_tensor("outT", [QSLICE, S], F32, kind="ExternalOutput").ap()

    xT_r = xT.rearrange("(kt p) s -> p kt s", p=128)
    wqT_r = wqT.rearrange("(kt p) m -> p kt m", p=128)
    wkvT_r = wkvT.rearrange("(kt p) m -> p kt m", p=128)
    woT_r = woT.rearrange("(kt p) m -> p kt m", p=128)

    with tile.TileContext(nc) as tc, ExitStack() as ctx:
        const = ctx.enter_context(tc.tile_pool(name="const", bufs=1))
        bigw = ctx.enter_context(tc.tile_pool(name="bigw", bufs=1))
        slab = ctx.enter_context(tc.tile_pool(name="slab", bufs=2))
        f32t = ctx.enter_context(tc.tile_pool(name="f32t", bufs=4))
        ppool = ctx.enter_context(tc.tile_pool(name="ppool", bufs=4))
        accp = ctx.enter_context(tc.tile_pool(name="accp", bufs=2))
        smalls = ctx.enter_context(tc.tile_pool(name="smalls", bufs=2))
        rinvp = ctx.enter_context(tc.tile_pool(name="rinvp", bufs=2))
        otp = ctx.enter_context(tc.tile_pool(name="otp", bufs=2))
        dram = ctx.enter_context(tc.tile_pool(name="dram", bufs=1, space="DRAM"))
        pbig = ctx.enter_context(tc.tile_pool(name="pbig", bufs=2, space="PSUM"))
        ppo = ctx.enter_context(tc.tile_pool(name="ppo", bufs=2, space="PSUM"))
        paux = ctx.enter_context(tc.tile_pool(name="paux", bufs=2, space="PSUM"))

        # ---- persistent constants
        ones_t = const.tile([128, 1], BF16)
        nc.vector.memset(ones_t[:], 1.0)
        ones_row = const.tile([1, 128], F32)
        nc.vector.memset(ones_row[:], 1.0)
        ident = const.tile([128, 128], BF16)
        make_identity(nc, ident[:])

        # mask bias tiles: mb[k, d, q] = 0 where q - 128d - k >= 0 else -1e9
        maskb = const.tile([128, 4, 512], BF16, name="maskb")
        nc.gpsimd.memset(maskb[:], 0.0)
        for d in range(4):
            nc.gpsimd.affine_select(
                maskb[:, d, :], maskb[:, d, :], pattern=[[1, 512]],
                compare_op=mybir.AluOpType.is_ge, fill=NEG,
                base=-128 * d, channel_multiplier=-1)

        cos_sb = const.tile([128, S], F32)
        nc.sync.dma_start(cos_sb[:], cosT[:])
        sin_sb = const.tile([128, S], F32)
        nc.sync.dma_start(sin_sb[:], sinT[:])

        qT_sb = const.tile([128, HEADS_PER_CORE, S], BF16)         # 16KB/part
        kT_sb = const.tile([128, S], BF16)                         # 4KB/part
        v_sb = const.tile([128, S // 128, HEAD_DIM], BF16)         # 4KB/part
        wkv_sb = const.tile([128, KT, 2 * HEAD_DIM], BF16)         # 16KB/part
        # wq and wo share one 32KB/part slot; wo loads once proj is done
        wq_sb = bigw.tile([128, KT, QSLICE], BF16, tag="bigw", name="wq_sb")

        ag_ins = [dram.tile([QSLICE, 512], BF16, tag=f"agin{j}",
                            name=f"agin{j}") for j in range(SC)]
        ag_outs = [dram.tile([NUM_HEADS * HEAD_DIM, 512], BF16,
                             addr_space="Shared", tag=f"agout{j}",
                             name=f"agout{j}") for j in range(SC)]

        # ---- initial DMA: kv weights + x chunk 0 + q weights, chunked so
        # ktile 0 lands early; wo afterwards (needed only at the tail).
        x_slab0 = slab.tile([128, KT, 512], BF16, tag="slab", name="x_slab0")
        for g in range(8):
            kts = ds(4 * g, 4)
            nc.sync.dma_start(wkv_sb[:, kts, :], wkvT_r[:, kts, :])
            nc.sync.dma_start(x_slab0[:, kts, :], xT_r[:, kts, 0:512])
            nc.sync.dma_start(wq_sb[:, kts, :], wqT_r[:, kts, :])

        def rope(dst, src, s):
            """dst = src*cos + rotate_half(src)*sin_signed for seq chunk s."""
            rot = f32t.tile([128, 512], F32, tag="f32t", name="rot")
            nc.vector.tensor_tensor(rot[0:64, :], src[64:128, :],
                                    sin_sb[0:64, ts(s, 512)],
                                    mybir.AluOpType.mult)
            nc.vector.tensor_tensor(rot[64:128, :], src[0:64, :],
                                    sin_sb[64:128, ts(s, 512)],
                                    mybir.AluOpType.mult)
            cq = f32t.tile([128, 512], F32, tag="f32t", name="cq")
            nc.vector.tensor_tensor(cq[:], src[:], cos_sb[:, ts(s, 512)],
                                    mybir.AluOpType.mult)
            nc.vector.tensor_tensor(dst, cq[:], rot[:], mybir.AluOpType.add)

        def proj_chunk(s, x_slab):
            # group 1: V into half 0, K into half 1 of one 2-bank psum tile
            pvk = pbig.tile([128, 2, 512], F32, tag="big", name="pvk")
            for kt in range(KT):
                nc.tensor.matmul(pvk[:, 0, :], wkv_sb[:, kt, ds(128, 128)],
                                 x_slab[:, kt, :],
                                 start=(kt == 0), stop=(kt == KT - 1))
            for kt in range(KT):
                nc.tensor.matmul(pvk[:, 1, :], wkv_sb[:, kt, ds(0, 128)],
                                 x_slab[:, kt, :],
                                 start=(kt == 0), stop=(kt == KT - 1))
            vt_c = smalls.tile([128, 512], BF16, tag="vt", name="vt_c")
            nc.scalar.copy(vt_c[:], pvk[:, 0, :])
            rope(kT_sb[:, ts(s, 512)], pvk[:, 1, :], s)
            for t in range(4):
                ptr = paux.tile([128, 128], BF16, tag="aux", name="ptr")
                nc.tensor.transpose(ptr[:], vt_c[:, ts(t, 128)], ident[:])
                nc.scalar.copy(v_sb[:, 4 * s + t, :], ptr[:])
            # groups 2/3: Q head pairs
            for hp in range(2):
                pq = pbig.tile([128, 2, 512], F32, tag="big", name="pq")
                for u in range(2):
                    h = 2 * hp + u
                    for kt in range(KT):
                        nc.tensor.matmul(pq[:, u, :],
                                         wq_sb[:, kt, ts(h, 128)],
                                         x_slab[:, kt, :],
                                         start=(kt == 0), stop=(kt == KT - 1))
                for u in range(2):
                    h = 2 * hp + u
                    rope(qT_sb[:, h, ts(s, 512)], pq[:, u, :], s)

        def attn_chunk(j):
            nunit = 2 * (j + 1)
            for h in range(HEADS_PER_CORE):
                po = ppo.tile([128, 512], F32, tag="po", name="po")
                acc = accp.tile([128, 512], BF16, tag="acc", name="acc")

                def emit_scores(u, h=h, j=j):
                    ps = pbig.tile([128, 2, 512], F32, tag="big", name="ps")
                    for w in range(2):
                        ki = 2 * u + w
                        d = ki - 4 * j
                        nc.tensor.matmul(ps[:, w, :], kT_sb[:, ts(ki, 128)],
                                         qT_sb[:, h, ts(j, 512)],
                                         start=True, stop=(d < 0))
                        if d >= 0:  # diagonal block: add -1e9 causal bias
                            nc.tensor.matmul(ps[:, w, :], ident[:],
                                             maskb[:, d, :],
                                             start=False, stop=True)
                    pT = ppool.tile([128, 2, 512], BF16, tag="pT", name="pT")
                    nc.scalar.activation(pT[:], ps[:],
                                         mybir.ActivationFunctionType.Exp)
                    # denominator accumulation on DVE (bf16, 2x mode)
                    if u == 0:
                        nc.vector.tensor_tensor(acc[:], pT[:, 0, :],
                                                pT[:, 1, :],
                                                mybir.AluOpType.add)
                    else:
                        for w in range(2):
                            nc.vector.tensor_tensor(acc[:], acc[:],
                                                    pT[:, w, :],
                                                    mybir.AluOpType.add)
                    return pT

                DEPTH = 1
                pts = [emit_scores(uu) for uu in range(min(DEPTH, nunit))]
                for u in range(nunit):
                    pT = pts[u]
                    if u + DEPTH < nunit:
                        pts.append(emit_scores(u + DEPTH))
                    for w in range(2):
                        ki = 2 * u + w
                        nc.tensor.matmul(po[:], v_sb[:, ki, :], pT[:, w, :],
                                         start=(ki == 0),
                                         stop=(ki == 2 * nunit - 1))
                # normalization: pd = ones @ acc; broadcast 1/pd via PE outer
                pd = paux.tile([1, 512], F32, tag="aux", name="pd")
                nc.tensor.matmul(pd[:], ones_t[:], acc[:], start=True,
                                 stop=True)
                pd_sb = smalls.tile([1, 512], F32, tag="pd", name="pd_sb")
                nc.scalar.copy(pd_sb[:], pd[:])
                rbraw = paux.tile([128, 512], F32, tag="aux", name="rbraw")
                nc.tensor.matmul(rbraw[:], ones_row[:], pd_sb[:], start=True,
                                 stop=True)
                rinv = rinvp.tile([128, 512], F32, tag="rinv", name="rinv")
                nc.vector.reciprocal_approx_fast(rinv[:], rbraw[:])
                att = smalls.tile([128, 512], BF16, tag="att", name="att")
                nc.vector.tensor_tensor(att[:], po[:], rinv[:],
                                        mybir.AluOpType.mult)
                nc.sync.dma_start(ag_ins[j][ts(h, 128), :], att[:])
            nc.gpsimd.collective_compute(
                "AllGather", mybir.AluOpType.bypass,
                replica_groups=[list(range(N_CORES))],
                ins=[ag_ins[j].opt()], outs=[ag_outs[j].opt()],
            )

        def oproj_chunk(s, wo_sb):
            ag_r = ag_outs[s].rearrange("(kt p) s -> p kt s", p=128)
            a_slab = slab.tile([128, KT, 512], BF16, tag="slab", name="a_slab")
            nc.sync.dma_start(a_slab[:], ag_r[:])
            for fp in range(2):
                pq = pbig.tile([128, 2, 512], F32, tag="big", name="pq_o")
                for u in range(2):
                    ft = 2 * fp + u
                    for kt in range(KT):
                        nc.tensor.matmul(pq[:, u, :],
                                         wo_sb[:, kt, ts(ft, 128)],
                                         a_slab[:, kt, :],
                                         start=(kt == 0), stop=(kt == KT - 1))
                ot = otp.tile([128, 2, 512], F32, tag="ot", name="ot")
                nc.scalar.copy(ot[:], pq[:])
                for u in range(2):
                    nc.sync.dma_start(outT[ds((2 * fp + u) * 128, 128),
                                           ts(s, 512)], ot[:, u, :])

        # ---- main schedule: proj/attention interleaved, o_proj at tail
        for s in range(SC):
            if s == 0:
                x_slab = x_slab0
            else:
                x_slab = slab.tile([128, KT, 512], BF16, tag="slab",
                                   name="x_slab")
                nc.sync.dma_start(x_slab[:], xT_r[:, :, ts(s, 512)])
            proj_chunk(s, x_slab)
            attn_chunk(s)
        wo_sb = bigw.tile([128, KT, QSLICE], BF16, tag="bigw", name="wo_sb")
        nc.sync.dma_start(wo_sb[:], woT_r[:])
        for s in range(SC):
            oproj_chunk(s, wo_sb)

    nc.finalize()
    return nc


def _prep_inputs(hidden_states, Wq, Wk, Wv, Wo, position_ids):
    """Slice/cast per-core inputs (host-side layout prep only)."""
    bf = ml_dtypes.bfloat16
    x = np.ascontiguousarray(np.asarray(hidden_states, np.float32)[0].T).astype(bf)
    scale = 1.0 / np.sqrt(HEAD_DIM)
    # rotary tables, [head_dim, seq]; sin signed (first half negated)
    invf_half = (1.0 / (ROPE_THETA ** (np.arange(0, HEAD_DIM, 2, dtype=np.float64)
                                       / HEAD_DIM)))
    invf = np.concatenate([invf_half, invf_half])  # [128]
    pos = np.asarray(position_ids, np.float64).reshape(S)
    ang = invf[:, None] * pos[None, :]             # [128, S]
    cosT = np.cos(ang).astype(np.float32)
    sinT = np.sin(ang).astype(np.float32)
    sinT[:HEAD_DIM // 2] *= -1.0
    in_maps = []
    for c in range(N_CORES):
        wq_c = (np.asarray(Wq, np.float32)[c * QSLICE:(c + 1) * QSLICE] * scale)
        wk_c = np.asarray(Wk, np.float32)[c * HEAD_DIM:(c + 1) * HEAD_DIM]
        wv_c = np.asarray(Wv, np.float32)[c * HEAD_DIM:(c + 1) * HEAD_DIM]
        wkv_c = np.concatenate([wk_c, wv_c], axis=0)   # [256, 4096]
        wo_c = np.asarray(Wo, np.float32)[c * QSLICE:(c + 1) * QSLICE]
        in_maps.append({
            "xT": x,
            "wqT": np.ascontiguousarray(wq_c.T).astype(bf),
            "wkvT": np.ascontiguousarray(wkv_c.T).astype(bf),
            "woT": np.ascontiguousarray(wo_c.T).astype(bf),
            "cosT": cosT,
            "sinT": sinT,
        })
    return in_maps


def kernel(hidden_states, Wq, Wk, Wv, Wo, position_ids):
    from concourse.bass_utils import run_bass_kernel_spmd
    if "nc" not in _cache:
        _cache["nc"] = build_nc()
    nc = _cache["nc"]
    in_maps = _prep_inputs(hidden_states, Wq, Wk, Wv, Wo, position_ids)
    res = run_bass_kernel_spmd(nc, in_maps, core_ids=list(range(N_CORES)))
    out = np.concatenate([res.results[c]["outT"].T for c in range(N_CORES)], axis=1)
    return out[None].astype(np.float32)
